# revision 8
# baseline (speedup 1.0000x reference)
"""Batched cosine-similarity matrix (retrieval_knn) on 8 TRN2 NeuronCores.

reference:  out[b, n, m] = <x[b,n,:], y[b,m,:]> / max(||x[b,n]|| * ||y[b,m]||, 1e-8)
shapes:     x, y: [8, 2048, 512] f32  ->  out: [8, 2048, 2048] f32

Sharding: data-parallel over the batch dim — batch b runs on core b.
Each core receives xT=x[b].T, yT=y[b].T and an untransposed copy xN=x[b],
all cast to bf16 host-side (bf16 halves DMA traffic and enables the PE's
fast weight load, which fp32/f32r weights cannot use).

Per-core kernel (all-bf16 matmuls, f32 PSUM accumulation):
  y-side: per 512-col block: ysq = y*y (DVE) -> sny = ones.T @ ysq (PE,
    replicated column sums) -> rsy = 1/sqrt (ACT sqrt + DVE
    reciprocal_approx_fast + bf16 cast; ACT Rsqrt is banned) ->
    yhat = y * rsy (DVE).  The column-norm scale rides in the moving
    matmul operand.
  x-side: row norms come from the untransposed xN copy: one DVE
    tensor_tensor_reduce per [128,512] row-tile accumulates ssq_x per
    PARTITION, i.e. directly in the per-partition layout the epilogue
    needs — no PE matmuls and no prescale for x at all.
  dots = xT.T @ yhat                (PE, 64 tiles [128,512], k=4x128)
  out  = dots * rx[n]               (per-partition scale + bf16 cast;
                                     alternates ACT activation-scale and
                                     DVE tensor_scalar so neither engine
                                     becomes the bottleneck)
A short burst of dummy matmuls on the constants tile warms the PE's HAM
activity window during the input DMA; DMA issue order is staged so every
operand lands just ahead of its consumer.  Output is written bf16 and
upcast to f32 host-side.
"""

import numpy as np
import ml_dtypes

import concourse.bass as bass
import concourse.bacc as bacc
import concourse.mybir as mybir
import concourse.tile as tile
from concourse.bass_utils import run_bass_kernel_spmd

P = 128          # partitions
D = 512          # feature dim (contraction)
N = 2048         # rows of x / y
B = 8            # batch == n_cores
KC = D // P      # 4 k-chunks
NT = N // P      # 16 n-tiles (output partition tiles)
MC = N // 512    # 4 m-chunks (output free chunks, PSUM-bank width)
WARMUP_MM = 24   # dummy [128,128] matmuls to warm the PE clock

F32 = mybir.dt.float32
BF16 = mybir.dt.bfloat16

_CACHED = {}


def _build_nc() -> bass.Bass:
    """Build the single-core Bass program (same program runs SPMD on 8 cores)."""
    nc = bacc.Bacc(trn_type="TRN2", target_bir_lowering=False, debug=False)

    xT = nc.dram_tensor("xT", [D, N], BF16, kind="ExternalInput").ap()
    yT = nc.dram_tensor("yT", [D, N], BF16, kind="ExternalInput").ap()
    xN = nc.dram_tensor("xN", [N, D], BF16, kind="ExternalInput").ap()
    out = nc.dram_tensor("out", [N, N], BF16, kind="ExternalOutput").ap()

    with tile.TileContext(nc) as tc:
        with (
            tc.tile_pool(name="xin", bufs=1) as xin_pool,
            tc.tile_pool(name="yin", bufs=1) as yin_pool,
            tc.tile_pool(name="xnd", bufs=1) as xnd_pool,
            tc.tile_pool(name="sq", bufs=1) as sq_pool,
            tc.tile_pool(name="consts", bufs=1) as const_pool,
            tc.tile_pool(name="norms", bufs=1) as norm_pool,
            tc.tile_pool(name="yh", bufs=1) as yh_pool,
            tc.tile_pool(name="ostage", bufs=6) as out_pool,
            tc.tile_pool(name="mm_ps", bufs=6, space="PSUM") as mm_ps_pool,
            tc.tile_pool(name="n_ps", bufs=2, space="PSUM") as n_ps_pool,
        ):
            ones_f = const_pool.tile([P, P], F32, name="ones_f")
            nc.vector.memset(ones_f, 1.0)
            ones = const_pool.tile([P, P], BF16, name="ones")
            nc.scalar.copy(ones, ones_f)

            # PE warm-up: keep the HAM activity window busy while the first
            # input blocks stream in, so real matmuls start at full clock.
            wps = n_ps_pool.tile([P, 512], F32, name="wps", tag="n_ps")
            for _ in range(WARMUP_MM):
                nc.tensor.matmul(wps[:, 0:P], lhsT=ones, rhs=ones,
                                 start=True, stop=True)

            # ---- tiles ------------------------------------------------
            xt = [xin_pool.tile([P, N], BF16, name=f"xt{k}", tag=f"xt{k}")
                  for k in range(KC)]
            yt = [yin_pool.tile([P, N], BF16, name=f"yt{k}", tag=f"yt{k}")
                  for k in range(KC)]
            xn = [xnd_pool.tile([P, D], BF16, name=f"xn{t}", tag=f"xn{t}")
                  for t in range(NT)]
            ysq = [sq_pool.tile([P, N], BF16, name=f"ysq{k}", tag=f"ysq{k}")
                   for k in range(KC)]
            yhat = [yh_pool.tile([P, N], BF16, name=f"yh{k}", tag=f"yh{k}")
                    for k in range(KC)]
            sny = norm_pool.tile([P, N], F32, name="sny")
            rsny_f = norm_pool.tile([P, N], F32, name="rsny_f")
            rsny = norm_pool.tile([P, N], BF16, name="rsny")
            ssqx = norm_pool.tile([P, NT], F32, name="ssqx")
            snxp = norm_pool.tile([P, NT], F32, name="snxp")
            rx = norm_pool.tile([P, NT], F32, name="rx")
            sqs = norm_pool.tile([P, D], BF16, name="sqs")    # ACT sq scratch
            sqs2 = norm_pool.tile([P, D], BF16, name="sqs2")  # DVE sq scratch

            # ---- input DMAs, staged by first-use time ----------------
            def dma_xT(b):
                cs = slice(b * 512, (b + 1) * 512)
                for k in range(KC):
                    nc.sync.dma_start(out=xt[k][:, cs], in_=xT[k * P:(k + 1) * P, cs])

            def dma_yT(b):
                cs = slice(b * 512, (b + 1) * 512)
                for k in range(KC):
                    nc.sync.dma_start(out=yt[k][:, cs], in_=yT[k * P:(k + 1) * P, cs])

            def dma_xN(q):
                for t in range(4 * q, 4 * q + 4):
                    nc.sync.dma_start(out=xn[t], in_=xN[t * P:(t + 1) * P, :])

            dma_xT(0); dma_yT(0); dma_xN(0); dma_xT(1); dma_xN(1)
            dma_xT(2); dma_xN(2); dma_yT(1); dma_xT(3); dma_xN(3)
            dma_yT(2); dma_yT(3)

            # ---- norm pipelines --------------------------------------
            def norm_y(b):
                """yhat = y/||y|| for 512-col block b."""
                cs = slice(b * 512, (b + 1) * 512)
                for k in range(KC):
                    nc.vector.tensor_tensor(ysq[k][:, cs], yt[k][:, cs],
                                            yt[k][:, cs], mybir.AluOpType.mult)
                n_ps = n_ps_pool.tile([P, 512], F32, name="n_ps", tag="n_ps")
                for k in range(KC):
                    nc.tensor.matmul(n_ps, lhsT=ones, rhs=ysq[k][:, cs],
                                     start=(k == 0), stop=(k == KC - 1))
                nc.scalar.sqrt(sny[:, cs], n_ps)
                nc.vector.reciprocal_approx_fast(rsny_f[:, cs], sny[:, cs])
                nc.vector.tensor_copy(rsny[:, cs], rsny_f[:, cs])
                for k in range(KC):
                    nc.vector.tensor_tensor(yhat[k][:, cs], yt[k][:, cs],
                                            rsny[:, cs], mybir.AluOpType.mult)

            def norm_x_quad(q):
                """rx[:, t] = 1/||x_row|| for row-tiles t=4q..4q+3 (per-partition).

                Sum-of-squares alternates ACT (Square + free-dim accum) and
                DVE (square + tensor_reduce) so neither engine eats the full
                cost.  (tensor_tensor_reduce crashes the device; avoided.)
                """
                qs = slice(4 * q, 4 * q + 4)
                for t in range(4 * q, 4 * q + 4):
                    if t % 2 == 0:
                        nc.scalar.activation(
                            sqs, xn[t], mybir.ActivationFunctionType.Square,
                            bias=0.0, scale=1.0, accum_out=ssqx[:, t:t + 1])
                    else:
                        nc.vector.tensor_tensor(sqs2, xn[t], xn[t],
                                                mybir.AluOpType.mult)
                        nc.vector.tensor_reduce(ssqx[:, t:t + 1], sqs2,
                                                mybir.AxisListType.X,
                                                mybir.AluOpType.add)
                nc.scalar.sqrt(snxp[:, qs], ssqx[:, qs])
                nc.vector.reciprocal_approx_fast(rx[:, qs], snxp[:, qs])

            norm_y(0)
            norm_x_quad(0)

            # ---- main loop -------------------------------------------
            for c in range(MC):
                cs = slice(c * 512, (c + 1) * 512)
                for t in range(NT):
                    if c == 0 and t in (2, 6, 10):
                        norm_x_quad(t // 4 + 1)
                    if t == 8 and c + 1 < MC:
                        norm_y(c + 1)
                    ts_ = slice(t * P, (t + 1) * P)
                    ps = mm_ps_pool.tile([P, 512], F32, name="ps", tag="ps")
                    for k in range(KC):
                        nc.tensor.matmul(
                            ps, lhsT=xt[k][:, ts_], rhs=yhat[k][:, cs],
                            start=(k == 0), stop=(k == KC - 1),
                        )
                    ot = out_pool.tile([P, 512], BF16, name="ot", tag="ot")
                    if t % 2 == 0:
                        nc.scalar.activation(ot, ps,
                                             mybir.ActivationFunctionType.Copy,
                                             bias=0.0, scale=rx[:, t:t + 1])
                    else:
                        nc.vector.tensor_scalar(ot, ps, rx[:, t:t + 1], None,
                                                mybir.AluOpType.mult)
                    nc.sync.dma_start(out=out[ts_, cs], in_=ot)

    nc.compile()
    return nc


def _get_nc() -> bass.Bass:
    if "bf16" not in _CACHED:
        _CACHED["bf16"] = _build_nc()
    return _CACHED["bf16"]


def _shard(x: np.ndarray, y: np.ndarray):
    """Host-side sharding: batch b -> core b; bf16 copies xT, yT, xN."""
    x = np.asarray(x, dtype=np.float32)
    y = np.asarray(y, dtype=np.float32)
    xNs = x.astype(ml_dtypes.bfloat16)
    xTs = np.ascontiguousarray(np.transpose(x, (0, 2, 1))).astype(ml_dtypes.bfloat16)
    yTs = np.ascontiguousarray(np.transpose(y, (0, 2, 1))).astype(ml_dtypes.bfloat16)
    return [{"xT": xTs[b], "yT": yTs[b], "xN": xNs[b]} for b in range(B)]


def _run(x: np.ndarray, y: np.ndarray, mm_dtype: str = "bf16",
         trace: bool = False):
    """Returns (out [8, 2048, 2048] f32, BassKernelResults)."""
    nc = _get_nc()
    in_maps = _shard(x, y)
    res = run_bass_kernel_spmd(nc, in_maps, core_ids=list(range(B)), trace=trace)
    out = np.stack([res.results[b]["out"].astype(np.float32) for b in range(B)])
    return out, res


def kernel(x: np.ndarray, y: np.ndarray) -> np.ndarray:
    out, _ = _run(x, y)
    return out
